# revision 1
# baseline (speedup 1.0000x reference)
"""FlowNetC-style correlation (cost volume) kernel for Trainium2.

Input : feat1, feat2  [B=8, H=128, W=256, C=128] fp32
Output: [B, H, W, 81]  -- out[b,h,w,dy*9+dx] = sum_c f1[b,h,w,c] * f2p[b,h+dy,w+dx,c]
        where f2p is feat2 zero-padded by 4 on each spatial side.

Per NeuronCore (batch-sharded, 1 image/core):
  - Build fp16 transposed copies via PE transpose (fp32 in, cast-to-fp16 on
    PSUM eviction): f1T blocks [C, 8*256] and padded f2pT [C, 136, 264].
  - Correlation per (h, w-half): 4 column-group matmuls (tile_position
    (0,32A), M=32 pixels, K=C=128, N=41*9=369). Group A's rhs is its own
    41-col x 9-row band of f2pT in j-major order, so
    psum[32A+m, j*9+dy] = corr(pixel w=wh*128+32A+m, dx=j-m, dy).
  - Evict psum [128,369] into E [128, 8*369] (8 h rows per block).
  - Extract the 81 useful values per pixel (contiguous run at element offset
    9*(p mod 32) of its 369-slot) with 32 pure-stride DMAs per E block.
    Run order per pixel is (dx,dy); the host swaps to (dy,dx).
"""

import sys

if '/opt/trn_rl_repo' not in sys.path:
    sys.path.insert(0, '/opt/trn_rl_repo')

import numpy as np

import concourse.bacc as bacc
import concourse.mybir as mybir
from concourse import masks
from concourse.ap import AP
from concourse.bass_utils import run_bass_kernel_spmd
from concourse.tile import TileContext

H, W, C = 128, 256, 128
D = 9                      # displacement window 9x9
HP, WP = H + 8, W + 8      # padded f2 spatial dims (136, 264)
JW = 40                    # band width per 32-pixel strip (32 + 9 - 1)
NW = JW * D                # 360 = matmul N per (h, w-half)
HB = 8                     # h rows batched per E block
ROW_E = HB * NW            # 2952 elements per partition per E block
F32 = mybir.dt.float32
F16 = mybir.dt.float16

_CACHED_NC = None


def _build():
    nc = bacc.Bacc("TRN2", target_bir_lowering=False, debug=False,
                   num_devices=1)
    f1_d = nc.dram_tensor("feat1", [H, W, C], F32, kind="ExternalInput")
    f2_d = nc.dram_tensor("feat2", [H, W, C], F32, kind="ExternalInput")
    # per-pixel 81-runs are (dx, dy)-ordered; host transposes to (dy, dx)
    out_d = nc.dram_tensor("out", [H, W, 81], F32, kind="ExternalOutput")

    with TileContext(nc) as tc:
        with (
            tc.tile_pool(name="const", bufs=1) as constp,
            tc.tile_pool(name="big", bufs=1) as bigp,
            tc.tile_pool(name="stag", bufs=4) as stagp,
            tc.tile_pool(name="f1t", bufs=16) as f1tp,
            tc.tile_pool(name="ebuf", bufs=2) as ep,
            tc.tile_pool(name="pst", bufs=2, space="PSUM") as pstp,
            tc.tile_pool(name="psc", bufs=3, space="PSUM") as pscp,
        ):
            ident = constp.tile([128, 128], F32)
            masks.make_identity(nc, ident[:, :])

            f2pT = bigp.tile([128, HP, WP], F16)     # 71.8KB/partition
            nc.vector.memset(f2pT[:, :, :], 0.0)

            f1t_blocks = []

            # ---- stage A: load -> PE transpose -> cast-evict to fp16 ----
            for blk in range(16):                    # 8 image rows per block
                h0 = blk * 8
                stag1 = stagp.tile([128, 16, 128], F32, tag="stag")
                stag2 = stagp.tile([128, 16, 128], F32, tag="stag")
                # one 1MB DMA per tensor; tile t=(hl,wh) -> stag[:, t, :]
                src1 = f1_d[h0:h0 + 8, :, :].rearrange(
                    "h (wt p) c -> p (h wt) c", p=128)
                nc.sync.dma_start(out=stag1[:, :, :], in_=src1)
                src2 = f2_d[h0:h0 + 8, :, :].rearrange(
                    "h (wt p) c -> p (h wt) c", p=128)
                nc.sync.dma_start(out=stag2[:, :, :], in_=src2)

                f1tb = f1tp.tile([128, 8 * 256], F16, tag="f1t")
                f1t_blocks.append(f1tb)
                for half in range(2):                # 8 transposes per bank-pair
                    pst = pstp.tile([128, 8, 128], F32, tag="pst")
                    for q in range(8):
                        t = half * 8 + q
                        nc.tensor.transpose(pst[:, q, :], stag1[:, t, :],
                                            ident[:, :])
                    dst1 = f1tb[:, half * 1024:(half + 1) * 1024]
                    src1 = pst[:, :, :].rearrange("c a b -> c (a b)")
                    if (blk + half) % 2 == 0:
                        nc.scalar.copy(dst1, src1)
                    else:
                        nc.vector.tensor_copy(dst1, src1)

                for half in range(2):
                    pst = pstp.tile([128, 8, 128], F32, tag="pst")
                    for q in range(8):
                        t = half * 8 + q
                        nc.tensor.transpose(pst[:, q, :], stag2[:, t, :],
                                            ident[:, :])
                    # tile t=(hl,wh): pixels -> f2pT row h0+4+t//2, col 4+128*(t%2)
                    hl0 = half * 4
                    dst = f2pT[:, h0 + 4 + hl0:h0 + 4 + hl0 + 4, 4:260]
                    dst = dst.rearrange("c a (b d) -> c a b d", b=2)
                    srcT = pst[:, :, :].rearrange("c (a b) d -> c a b d", b=2)
                    if (blk + half) % 2 == 0:
                        nc.vector.tensor_copy(dst, srcT)
                    else:
                        nc.scalar.copy(dst, srcT)

            # ---- stage B: correlation + eviction + extraction ----
            for hblk in range(H // HB):
                for wh in range(2):
                    E = ep.tile([128, ROW_E], F32, tag="ebuf")
                    for hl in range(HB):
                        h = hblk * HB + hl
                        ps = pscp.tile([128, NW], F32, tag="psc")
                        f1tb = f1t_blocks[h // 8]
                        r = h % 8
                        base = r * 256 + wh * 128
                        for A in range(4):
                            lhsT = f1tb[:, base + 32 * A:base + 32 * A + 32]
                            w0 = wh * 128 + 32 * A
                            rhs = f2pT[:, h:h + D, w0:w0 + JW].rearrange(
                                "c a b -> c b a")      # j-major
                            nc.tensor.matmul(
                                ps[32 * A:32 * A + 32, :], lhsT, rhs,
                                start=True, stop=True,
                                tile_position=(0, 32 * A))
                        if hl % 2 == 0:
                            nc.scalar.copy(E[:, hl * NW:(hl + 1) * NW],
                                           ps[:, :])
                        else:
                            nc.vector.tensor_copy(
                                E[:, hl * NW:(hl + 1) * NW], ps[:, :])
                    # 32 pure-stride extraction DMAs
                    h0 = hblk * HB
                    for f in range(32):
                        src = AP(tensor=E[:, :].tensor,
                                 offset=f * ROW_E + 9 * f,
                                 ap=[[32 * ROW_E, 4], [NW, HB], [1, 81]])
                        dst = AP(tensor=out_d,
                                 offset=(h0 * W + wh * 128 + f) * 81,
                                 ap=[[32 * 81, 4], [W * 81, HB], [1, 81]])
                        nc.sync.dma_start(out=dst, in_=src)

    nc.compile()
    return nc


def kernel(feat1: np.ndarray, feat2: np.ndarray) -> np.ndarray:
    global _CACHED_NC
    feat1 = np.ascontiguousarray(np.asarray(feat1), dtype=np.float32)
    feat2 = np.ascontiguousarray(np.asarray(feat2), dtype=np.float32)
    B = feat1.shape[0]
    if _CACHED_NC is None:
        _CACHED_NC = _build()
    nc = _CACHED_NC
    in_maps = [{"feat1": feat1[b], "feat2": feat2[b]} for b in range(B)]
    res = run_bass_kernel_spmd(nc, in_maps, core_ids=list(range(B)))
    outs = [res.results[b]["out"] for b in range(B)]
    out = np.stack(outs, axis=0)                       # [B,H,W,81] (dx,dy)
    out = out.reshape(B, H, W, 9, 9).swapaxes(-1, -2)  # -> (dy,dx)
    return np.ascontiguousarray(out.reshape(B, H, W, 81))



# revision 3
# speedup vs baseline: 2.5595x; 2.5595x over previous
"""FlowNetC-style correlation (cost volume) kernel for Trainium2.

Input : feat1, feat2  [B=8, H=128, W=256, C=128] fp32
Output: [B, H, W, 81]  -- out[b,h,w,dy*9+dx] = sum_c f1[b,h,w,c] * f2p[b,h+dy,w+dx,c]
        where f2p is feat2 zero-padded by 4 on each spatial side.

Per NeuronCore (batch-sharded, 1 image/core):
  - Build fp16 transposed copies via PE transpose (fp32 in, cast-to-fp16 on
    PSUM eviction): padded f2pT [C, 136, 264] up front; f1T blocks [C, 8*256]
    rolling (one 8-row block in flight), interleaved with correlation.
  - Correlation per (h, w-half): 4 column-group matmuls (tile_position
    (0,32A), M=32 pixels, K=C=128, N=40*9=360). Group A's rhs is its own
    40-col x 9-row band of f2pT in j-major order, so
    psum[32A+m, j*9+dy] = corr(pixel w=wh*128+32A+m, dx=j-m, dy).
  - Evict psum [128,360] as fp16 into E accumulation tiles [128, 2*8*360]
    (two 8-row blocks per tile), then DMA each full E tile to HBM as one
    big contiguous transfer (128 descriptors of 11.25KB).
  - The host extracts each pixel's 81 useful values (contiguous run at
    element offset 9*(p mod 32) of its 360-slot) with a strided view, and
    swaps the per-pixel run order (dx,dy) -> (dy,dx).
"""

import sys

if '/opt/trn_rl_repo' not in sys.path:
    sys.path.insert(0, '/opt/trn_rl_repo')

import numpy as np

import concourse.bacc as bacc
import concourse.mybir as mybir
from concourse import masks
from concourse.bass_utils import run_bass_kernel_spmd
from concourse.tile import TileContext

H, W, C = 128, 256, 128
D = 9                      # displacement window 9x9
HP, WP = H + 8, W + 8      # padded f2 spatial dims (136, 264)
JW = 40                    # band width per 32-pixel strip (32 + 9 - 1)
NW = JW * D                # 360 = matmul N per (h, w-half)
GB = 2                     # h-blocks (of 8 rows) per E tile / output dump
ROW_E = GB * 8 * NW        # 5760 elements per partition per E tile
NG = 16 // GB              # 8 dump groups
F32 = mybir.dt.float32
F16 = mybir.dt.float16

_CACHED_NC = None


def _build():
    nc = bacc.Bacc("TRN2", target_bir_lowering=False, debug=False,
                   num_devices=1)
    f1_d = nc.dram_tensor("feat1", [H, W, C], F32, kind="ExternalInput")
    f2_d = nc.dram_tensor("feat2", [H, W, C], F32, kind="ExternalInput")
    # Raw band output [wh, g, part, GB*8*360] fp16; host extracts the
    # 81-run per pixel and reorders.
    out_d = nc.dram_tensor("out", [2, NG, 128, ROW_E], F16,
                           kind="ExternalOutput")

    with TileContext(nc) as tc:
        with (
            tc.tile_pool(name="const", bufs=1) as constp,
            tc.tile_pool(name="big", bufs=1) as bigp,
            tc.tile_pool(name="stag", bufs=3) as stagp,
            tc.tile_pool(name="f1t", bufs=3) as f1tp,
            tc.tile_pool(name="ebuf", bufs=4) as ep,
            tc.tile_pool(name="pst", bufs=2, space="PSUM") as pstp,
            tc.tile_pool(name="psc", bufs=3, space="PSUM") as pscp,
        ):
            ident = constp.tile([128, 128], F32)
            masks.make_identity(nc, ident[:, :])

            f2pT = bigp.tile([128, HP, WP], F16)     # 71.8KB/partition
            # zero only the pad border; interior is fully overwritten
            nc.vector.memset(f2pT[:, 0:4, :], 0.0)
            nc.vector.memset(f2pT[:, HP - 4:HP, :], 0.0)
            nc.vector.memset(f2pT[:, 4:HP - 4, 0:4], 0.0)
            nc.vector.memset(f2pT[:, 4:HP - 4, WP - 4:WP], 0.0)

            # ---- stage A: f2 load -> PE transpose -> cast-evict to fp16 ----
            for blk in range(16):                    # 8 image rows per block
                h0 = blk * 8
                stag2 = stagp.tile([128, 16, 128], F32, tag="stag")
                src2 = f2_d[h0:h0 + 8, :, :].rearrange(
                    "h (wt p) c -> p (h wt) c", p=128)
                nc.sync.dma_start(out=stag2[:, :, :], in_=src2)

                for half in range(2):
                    pst = pstp.tile([128, 8, 128], F32, tag="pst")
                    for q in range(8):
                        t = half * 8 + q
                        nc.tensor.transpose(pst[:, q, :], stag2[:, t, :],
                                            ident[:, :])
                    # tile t=(hl,wh): pixels -> f2pT row h0+4+t//2, col 4+128*(t%2)
                    hl0 = half * 4
                    dst = f2pT[:, h0 + 4 + hl0:h0 + 4 + hl0 + 4, 4:260]
                    dst = dst.rearrange("c a (b d) -> c a b d", b=2)
                    srcT = pst[:, :, :].rearrange("c (a b) d -> c a b d", b=2)
                    if (blk + half) % 2 == 0:
                        nc.vector.tensor_copy(dst, srcT)
                    else:
                        nc.scalar.copy(dst, srcT)

            # ---- stage B: f1 load/transpose + correlation, interleaved ----
            e_tiles = {}
            for hblk in range(16):
                h0 = hblk * 8
                g = hblk // GB
                blkL = hblk % GB
                if blkL == 0:
                    for wh in range(2):
                        e_tiles[wh] = ep.tile([128, ROW_E], F16, tag="ebuf",
                                              name=f"E_{wh}_{g}")

                stag1 = stagp.tile([128, 16, 128], F32, tag="stag")
                src1 = f1_d[h0:h0 + 8, :, :].rearrange(
                    "h (wt p) c -> p (h wt) c", p=128)
                nc.sync.dma_start(out=stag1[:, :, :], in_=src1)

                f1tb = f1tp.tile([128, 8 * 256], F16, tag="f1t")
                for half in range(2):
                    pst = pstp.tile([128, 8, 128], F32, tag="pst")
                    for q in range(8):
                        t = half * 8 + q
                        nc.tensor.transpose(pst[:, q, :], stag1[:, t, :],
                                            ident[:, :])
                    dst1 = f1tb[:, half * 1024:(half + 1) * 1024]
                    src1T = pst[:, :, :].rearrange("c a b -> c (a b)")
                    if (hblk + half) % 2 == 0:
                        nc.scalar.copy(dst1, src1T)
                    else:
                        nc.vector.tensor_copy(dst1, src1T)

                for wh in range(2):
                    E = e_tiles[wh]
                    for hl in range(8):
                        h = h0 + hl
                        ps = pscp.tile([128, NW], F32, tag="psc")
                        base = hl * 256 + wh * 128
                        for A in range(4):
                            lhsT = f1tb[:, base + 32 * A:base + 32 * A + 32]
                            w0 = wh * 128 + 32 * A
                            rhs = f2pT[:, h:h + D, w0:w0 + JW].rearrange(
                                "c a b -> c b a")      # j-major
                            nc.tensor.matmul(
                                ps[32 * A:32 * A + 32, :], lhsT, rhs,
                                start=True, stop=True,
                                tile_position=(0, 32 * A))
                        dst = E[:, (blkL * 8 + hl) * NW:
                                (blkL * 8 + hl + 1) * NW]
                        if hl % 2 == 0:
                            nc.scalar.copy(dst, ps[:, :])
                        else:
                            nc.vector.tensor_copy(dst, ps[:, :])

                if blkL == GB - 1:
                    # one big contiguous dump per (wh, g): 128 x 11.25KB
                    for wh in range(2):
                        nc.sync.dma_start(out=out_d[wh, g, :, :],
                                          in_=e_tiles[wh][:, :])

    nc.compile()
    return nc


def _extract_host(raw: np.ndarray) -> np.ndarray:
    """raw [2, NG, 128, ROW_E] fp16 -> out [H, W, 81] fp32 (dy,dx order)."""
    arr = np.ascontiguousarray(raw).reshape(2, NG, 4, 32, GB, 8, NW)
    s = arr.strides
    # D[wh, g, pg, m, blkL, hl, k] = arr[wh, g, pg, m, blkL, hl, 9*m + k]
    diag = np.lib.stride_tricks.as_strided(
        arr,
        shape=(2, NG, 4, 32, GB, 8, 81),
        strides=(s[0], s[1], s[2], s[3] + 9 * s[6], s[4], s[5], s[6]),
    )
    # h = g*16 + blkL*8 + hl ; w = wh*128 + pg*32 + m ; k = dx*9 + dy
    out = diag.transpose(1, 4, 5, 0, 2, 3, 6).reshape(H, W, 81)
    out = out.reshape(H, W, 9, 9).swapaxes(-1, -2)   # -> (dy, dx)
    return np.ascontiguousarray(out.reshape(H, W, 81)).astype(np.float32)


def kernel(feat1: np.ndarray, feat2: np.ndarray) -> np.ndarray:
    global _CACHED_NC
    feat1 = np.ascontiguousarray(np.asarray(feat1), dtype=np.float32)
    feat2 = np.ascontiguousarray(np.asarray(feat2), dtype=np.float32)
    B = feat1.shape[0]
    if _CACHED_NC is None:
        _CACHED_NC = _build()
    nc = _CACHED_NC
    in_maps = [{"feat1": feat1[b], "feat2": feat2[b]} for b in range(B)]
    res = run_bass_kernel_spmd(nc, in_maps, core_ids=list(range(B)))
    out = np.stack([_extract_host(res.results[b]["out"]) for b in range(B)],
                   axis=0)
    return out


# revision 5
# speedup vs baseline: 3.5642x; 1.3925x over previous
"""FlowNetC-style correlation (cost volume) kernel for Trainium2.

Input : feat1, feat2  [B=8, H=128, W=256, C=128] fp32
Output: [B, H, W, 81]  -- out[b,h,w,dy*9+dx] = sum_c f1[b,h,w,c] * f2p[b,h+dy,w+dx,c]
        where f2p is feat2 zero-padded by 4 on each spatial side.

Per NeuronCore (batch-sharded, 1 image/core):
  - Build fp16 transposed copies via PE transpose (fp32 in, cast-to-fp16 on
    PSUM eviction): padded f2pT [C, 136, 264] up front; f1T blocks [C, 8*256]
    rolling (one 8-row block in flight), interleaved with correlation.
  - Correlation per (h, w-half): 4 column-group matmuls (tile_position
    (0,32A), M=32 pixels, K=C=128, N=40*9=360). Group A's rhs is its own
    40-col x 9-row band of f2pT in j-major order, so
    psum[32A+m, j*9+dy] = corr(pixel w=wh*128+32A+m, dx=j-m, dy).
  - Evict psum [128,360] as fp16 into E accumulation tiles [128, 2*8*360]
    (two 8-row blocks per tile), then DMA each full E tile to HBM as one
    big contiguous transfer (128 descriptors of 11.25KB).
  - The host extracts each pixel's 81 useful values (contiguous run at
    element offset 9*(p mod 32) of its 360-slot) with a strided view, and
    swaps the per-pixel run order (dx,dy) -> (dy,dx).
"""

import sys

if '/opt/trn_rl_repo' not in sys.path:
    sys.path.insert(0, '/opt/trn_rl_repo')

import numpy as np

import concourse.bacc as bacc
import concourse.mybir as mybir
from concourse import masks
from concourse.bass_utils import run_bass_kernel_spmd
from concourse.tile import TileContext

H, W, C = 128, 256, 128
D = 9                      # displacement window 9x9
HP, WP = H + 8, W + 8      # padded f2 spatial dims (136, 264)
JW = 40                    # band width per 32-pixel strip (32 + 9 - 1)
NW = JW * D                # 360 = matmul N per (h, w-half)
GB = 2                     # h-blocks (of 8 rows) per E tile / output dump
ROW_E = GB * 8 * NW        # 5760 elements per partition per E tile
NG = 16 // GB              # 8 dump groups
F32 = mybir.dt.float32
F16 = mybir.dt.float16

_CACHED_NC = None


def _build():
    nc = bacc.Bacc("TRN2", target_bir_lowering=False, debug=False,
                   num_devices=1)
    f1_d = nc.dram_tensor("feat1", [H, W, C], F32, kind="ExternalInput")
    f2_d = nc.dram_tensor("feat2", [H, W, C], F32, kind="ExternalInput")
    # Raw band output [wh, g, part, GB*8*360] fp16; host extracts the
    # 81-run per pixel and reorders.
    out_d = nc.dram_tensor("out", [2, NG, 128, ROW_E], F16,
                           kind="ExternalOutput")

    with TileContext(nc) as tc:
        with (
            tc.tile_pool(name="const", bufs=1) as constp,
            tc.tile_pool(name="big", bufs=1) as bigp,
            tc.tile_pool(name="stag", bufs=3) as stagp,
            tc.tile_pool(name="f1t", bufs=3) as f1tp,
            tc.tile_pool(name="ebuf", bufs=4) as ep,
            tc.tile_pool(name="pst", bufs=2, space="PSUM") as pstp,
            tc.tile_pool(name="psc", bufs=3, space="PSUM") as pscp,
        ):
            ident = constp.tile([128, 128], F32)
            masks.make_identity(nc, ident[:, :])

            f2pT = bigp.tile([128, HP, WP], F16)     # 71.8KB/partition
            # zero only the pad border; interior is fully overwritten
            nc.vector.memset(f2pT[:, 0:4, :], 0.0)
            nc.vector.memset(f2pT[:, HP - 4:HP, :], 0.0)
            nc.vector.memset(f2pT[:, 4:HP - 4, 0:4], 0.0)
            nc.vector.memset(f2pT[:, 4:HP - 4, WP - 4:WP], 0.0)

            # ---- stage A: f2 load -> PE transpose -> cast-evict to fp16 ----
            for blk in range(16):                    # 8 image rows per block
                h0 = blk * 8
                stag2 = stagp.tile([128, 16, 128], F32, tag="stag")
                src2 = f2_d[h0:h0 + 8, :, :].rearrange(
                    "h (wt p) c -> p (h wt) c", p=128)
                nc.sync.dma_start(out=stag2[:, :, :], in_=src2)

                for half in range(2):
                    pst = pstp.tile([128, 8, 128], F32, tag="pst")
                    for q in range(8):
                        t = half * 8 + q
                        nc.tensor.transpose(pst[:, q, :], stag2[:, t, :],
                                            ident[:, :])
                    # tile t=(hl,wh): pixels -> f2pT row h0+4+t//2, col 4+128*(t%2)
                    hl0 = half * 4
                    dst = f2pT[:, h0 + 4 + hl0:h0 + 4 + hl0 + 4, 4:260]
                    dst = dst.rearrange("c a (b d) -> c a b d", b=2)
                    srcT = pst[:, :, :].rearrange("c (a b) d -> c a b d", b=2)
                    if (blk + half) % 2 == 0:
                        nc.vector.tensor_copy(dst, srcT)
                    else:
                        nc.scalar.copy(dst, srcT)

            # ---- stage B: f1 load/transpose + correlation, interleaved ----
            e_tiles = {}
            for hblk in range(16):
                h0 = hblk * 8
                g = hblk // GB
                blkL = hblk % GB
                if blkL == 0:
                    for wh in range(2):
                        e_tiles[wh] = ep.tile([128, ROW_E], F16, tag="ebuf",
                                              name=f"E_{wh}_{g}")

                stag1 = stagp.tile([128, 16, 128], F32, tag="stag")
                src1 = f1_d[h0:h0 + 8, :, :].rearrange(
                    "h (wt p) c -> p (h wt) c", p=128)
                nc.sync.dma_start(out=stag1[:, :, :], in_=src1)

                f1tb = f1tp.tile([128, 8 * 256], F16, tag="f1t")
                for half in range(2):
                    pst = pstp.tile([128, 8, 128], F32, tag="pst")
                    for q in range(8):
                        t = half * 8 + q
                        nc.tensor.transpose(pst[:, q, :], stag1[:, t, :],
                                            ident[:, :])
                    dst1 = f1tb[:, half * 1024:(half + 1) * 1024]
                    src1T = pst[:, :, :].rearrange("c a b -> c (a b)")
                    if (hblk + half) % 2 == 0:
                        nc.scalar.copy(dst1, src1T)
                    else:
                        nc.vector.tensor_copy(dst1, src1T)

                for wh in range(2):
                    E = e_tiles[wh]
                    for hl in range(8):
                        h = h0 + hl
                        ps = pscp.tile([128, NW], F32, tag="psc")
                        base = hl * 256 + wh * 128
                        for A in range(4):
                            lhsT = f1tb[:, base + 32 * A:base + 32 * A + 32]
                            w0 = wh * 128 + 32 * A
                            # dy-major, j-contiguous: n = dy*JW + j
                            rhs = f2pT[:, h:h + D, w0:w0 + JW]
                            nc.tensor.matmul(
                                ps[32 * A:32 * A + 32, :], lhsT, rhs,
                                start=True, stop=True,
                                tile_position=(0, 32 * A))
                        dst = E[:, (blkL * 8 + hl) * NW:
                                (blkL * 8 + hl + 1) * NW]
                        if hl % 2 == 0:
                            nc.scalar.copy(dst, ps[:, :])
                        else:
                            nc.vector.tensor_copy(dst, ps[:, :])

                if blkL == GB - 1:
                    # one big contiguous dump per (wh, g): 128 x 11.25KB
                    for wh in range(2):
                        nc.sync.dma_start(out=out_d[wh, g, :, :],
                                          in_=e_tiles[wh][:, :])

    nc.compile()
    return nc


def _extract_host(raw: np.ndarray) -> np.ndarray:
    """raw [2, NG, 128, ROW_E] fp16 -> out [H, W, 81] fp32 (dy,dx order)."""
    arr = np.ascontiguousarray(raw).reshape(2, NG, 4, 32, GB, 8, NW)
    s = arr.strides
    # n = dy*JW + j with j = m + dx:
    # D[wh, g, pg, m, blkL, hl, dy, dx] = arr[..., m, blkL, hl, dy*JW + m + dx]
    diag = np.lib.stride_tricks.as_strided(
        arr,
        shape=(2, NG, 4, 32, GB, 8, 9, 9),
        strides=(s[0], s[1], s[2], s[3] + s[6], s[4], s[5],
                 JW * s[6], s[6]),
    )
    # h = g*16 + blkL*8 + hl ; w = wh*128 + pg*32 + m ; native (dy, dx)
    out = diag.transpose(1, 4, 5, 0, 2, 3, 6, 7).reshape(H, W, 81)
    return np.ascontiguousarray(out).astype(np.float32)


def kernel(feat1: np.ndarray, feat2: np.ndarray) -> np.ndarray:
    global _CACHED_NC
    feat1 = np.ascontiguousarray(np.asarray(feat1), dtype=np.float32)
    feat2 = np.ascontiguousarray(np.asarray(feat2), dtype=np.float32)
    B = feat1.shape[0]
    if _CACHED_NC is None:
        _CACHED_NC = _build()
    nc = _CACHED_NC
    in_maps = [{"feat1": feat1[b], "feat2": feat2[b]} for b in range(B)]
    res = run_bass_kernel_spmd(nc, in_maps, core_ids=list(range(B)))
    out = np.stack([_extract_host(res.results[b]["out"]) for b in range(B)],
                   axis=0)
    return out


# revision 8
# speedup vs baseline: 3.7444x; 1.0506x over previous
"""FlowNetC-style correlation (cost volume) kernel for Trainium2.

Input : feat1, feat2  [B=8, H=128, W=256, C=128] fp32
Output: [B, H, W, 81]  -- out[b,h,w,dy*9+dx] = sum_c f1[b,h,w,c] * f2p[b,h+dy,w+dx,c]
        where f2p is feat2 zero-padded by 4 on each spatial side.

Per NeuronCore (batch-sharded, 1 image/core):
  - Build fp16 transposed copies via PE transpose (fp32 in, cast-to-fp16 on
    PSUM eviction): padded f2pT [C, 136, 264] up front; f1T blocks [C, 8*256]
    rolling (one 8-row block in flight), interleaved with correlation.
  - Correlation per (h, w-half): 4 column-group matmuls (tile_position
    (0,32A), M=32 pixels, K=C=128, N=40*9=360). Group A's rhs is its own
    40-col x 9-row band of f2pT in j-major order, so
    psum[32A+m, j*9+dy] = corr(pixel w=wh*128+32A+m, dx=j-m, dy).
  - Evict psum [128,360] as fp16 into E accumulation tiles [128, 2*8*360]
    (two 8-row blocks per tile), then DMA each full E tile to HBM as one
    big contiguous transfer (128 descriptors of 11.25KB).
  - The host extracts each pixel's 81 useful values (contiguous run at
    element offset 9*(p mod 32) of its 360-slot) with a strided view, and
    swaps the per-pixel run order (dx,dy) -> (dy,dx).
"""

import sys

if '/opt/trn_rl_repo' not in sys.path:
    sys.path.insert(0, '/opt/trn_rl_repo')

import numpy as np

import concourse.bacc as bacc
import concourse.mybir as mybir
from concourse import masks
from concourse.bass_utils import run_bass_kernel_spmd
from concourse.tile import TileContext

H, W, C = 128, 256, 128
D = 9                      # displacement window 9x9
HP, WP = H + 8, W + 8      # padded f2 spatial dims (136, 264)
JW = 40                    # band width per 32-pixel strip (32 + 9 - 1)
NW = JW * D                # 360 = matmul N per (h, w-half)
GB = 1                     # h-blocks (of 8 rows) per E tile / output dump
ROW_E = GB * 8 * NW        # 2880 elements per partition per E tile
NG = 16 // GB              # 16 dump groups
F32 = mybir.dt.float32
F16 = mybir.dt.float16

_CACHED_NC = None


def _build():
    nc = bacc.Bacc("TRN2", target_bir_lowering=False, debug=False,
                   num_devices=1)
    f1_d = nc.dram_tensor("feat1", [H, W, C], F32, kind="ExternalInput")
    f2_d = nc.dram_tensor("feat2", [H, W, C], F32, kind="ExternalInput")
    # Raw band output [wh, g, part, GB*8*360] fp16; host extracts the
    # 81-run per pixel and reorders.
    out_d = nc.dram_tensor("out", [2, NG, 128, ROW_E], F16,
                           kind="ExternalOutput")

    with TileContext(nc) as tc:
        with (
            tc.tile_pool(name="const", bufs=1) as constp,
            tc.tile_pool(name="big", bufs=1) as bigp,
            tc.tile_pool(name="stag", bufs=3) as stagp,
            tc.tile_pool(name="f1t", bufs=3) as f1tp,
            tc.tile_pool(name="ebuf", bufs=4) as ep,
            tc.tile_pool(name="pst", bufs=2, space="PSUM") as pstp,
            tc.tile_pool(name="psc", bufs=3, space="PSUM") as pscp,
        ):
            ident = constp.tile([128, 128], F32)
            masks.make_identity(nc, ident[:, :])

            f2pT = bigp.tile([128, HP, WP], F16)     # 71.8KB/partition
            # zero only the pad border; interior is fully overwritten
            nc.vector.memset(f2pT[:, 0:4, :], 0.0)
            nc.vector.memset(f2pT[:, HP - 4:HP, :], 0.0)
            nc.vector.memset(f2pT[:, 4:HP - 4, 0:4], 0.0)
            nc.vector.memset(f2pT[:, 4:HP - 4, WP - 4:WP], 0.0)

            # ---- pipelined: load/transpose block blk ; correlate blk-1 ----
            f1t_tiles = {}

            def load_block(blk):
                h0 = blk * 8
                stag2 = stagp.tile([128, 16, 128], F32, tag="stag2",
                                   name=f"s2_{blk}")
                src2 = f2_d[h0:h0 + 8, :, :].rearrange(
                    "h (wt p) c -> p (h wt) c", p=128)
                nc.scalar.dma_start(out=stag2[:, :, :], in_=src2)

                stag1 = stagp.tile([128, 16, 128], F32, tag="stag1",
                                   name=f"s1_{blk}")
                src1 = f1_d[h0:h0 + 8, :, :].rearrange(
                    "h (wt p) c -> p (h wt) c", p=128)
                nc.sync.dma_start(out=stag1[:, :, :], in_=src1)

                for half in range(2):
                    pst = pstp.tile([128, 8, 128], F32, tag="pst")
                    for q in range(8):
                        t = half * 8 + q
                        nc.tensor.transpose(pst[:, q, :], stag2[:, t, :],
                                            ident[:, :])
                    # tile t=(hl,wh): pixels -> f2pT row h0+4+t//2, col 4+128*(t%2)
                    hl0 = half * 4
                    dst = f2pT[:, h0 + 4 + hl0:h0 + 4 + hl0 + 4, 4:260]
                    dst = dst.rearrange("c a (b d) -> c a b d", b=2)
                    srcT = pst[:, :, :].rearrange("c (a b) d -> c a b d", b=2)
                    if (blk + half) % 2 == 0:
                        nc.vector.tensor_copy(dst, srcT)
                    else:
                        nc.scalar.copy(dst, srcT)

                f1tb = f1tp.tile([128, 8 * 256], F16, tag="f1t",
                                 name=f"f1t_{blk}")
                f1t_tiles[blk] = f1tb
                for half in range(2):
                    pst = pstp.tile([128, 8, 128], F32, tag="pst")
                    for q in range(8):
                        t = half * 8 + q
                        nc.tensor.transpose(pst[:, q, :], stag1[:, t, :],
                                            ident[:, :])
                    dst1 = f1tb[:, half * 1024:(half + 1) * 1024]
                    src1T = pst[:, :, :].rearrange("c a b -> c (a b)")
                    if (blk + half) % 2 == 0:
                        nc.scalar.copy(dst1, src1T)
                    else:
                        nc.vector.tensor_copy(dst1, src1T)

            e_tiles = {}

            def corr_rows(hblk, hl_lo, hl_hi):
                h0 = hblk * 8
                f1tb = f1t_tiles[hblk]
                for wh in range(2):
                    if hl_lo == 0:
                        e_tiles[wh, hblk] = ep.tile(
                            [128, ROW_E], F16, tag="ebuf",
                            name=f"E_{wh}_{hblk}")
                    E = e_tiles[wh, hblk]
                    for hl in range(hl_lo, hl_hi):
                        h = h0 + hl
                        ps = pscp.tile([128, NW], F32, tag="psc")
                        base = hl * 256 + wh * 128
                        for A in range(4):
                            lhsT = f1tb[:, base + 32 * A:base + 32 * A + 32]
                            w0 = wh * 128 + 32 * A
                            # dy-major, j-contiguous: n = dy*JW + j
                            rhs = f2pT[:, h:h + D, w0:w0 + JW]
                            nc.tensor.matmul(
                                ps[32 * A:32 * A + 32, :], lhsT, rhs,
                                start=True, stop=True,
                                tile_position=(0, 32 * A))
                        dst = E[:, hl * NW:(hl + 1) * NW]
                        if hl % 2 == 0:
                            nc.scalar.copy(dst, ps[:, :])
                        else:
                            nc.vector.tensor_copy(dst, ps[:, :])
                    if hl_hi == 8:
                        # one contiguous dump per (wh, hblk): 128 x 5.6KB
                        nc.sync.dma_start(out=out_d[wh, hblk, :, :],
                                          in_=e_tiles.pop((wh, hblk))[:, :])

            load_block(0)
            for blk in range(1, 16):
                # rows hl<4 of block blk-1 only need f2 blocks <= blk-1
                corr_rows(blk - 1, 0, 4)
                load_block(blk)
                corr_rows(blk - 1, 4, 8)
                f1t_tiles.pop(blk - 1)
            corr_rows(15, 0, 4)
            corr_rows(15, 4, 8)
            f1t_tiles.pop(15)

    nc.compile()
    return nc


def _extract_host(raw: np.ndarray) -> np.ndarray:
    """raw [2, NG, 128, ROW_E] fp16 -> out [H, W, 81] fp32 (dy,dx order)."""
    arr = np.ascontiguousarray(raw).reshape(2, NG, 4, 32, GB, 8, NW)
    s = arr.strides
    # n = dy*JW + j with j = m + dx:
    # D[wh, g, pg, m, blkL, hl, dy, dx] = arr[..., m, blkL, hl, dy*JW + m + dx]
    diag = np.lib.stride_tricks.as_strided(
        arr,
        shape=(2, NG, 4, 32, GB, 8, 9, 9),
        strides=(s[0], s[1], s[2], s[3] + s[6], s[4], s[5],
                 JW * s[6], s[6]),
    )
    # h = g*16 + blkL*8 + hl ; w = wh*128 + pg*32 + m ; native (dy, dx)
    out = diag.transpose(1, 4, 5, 0, 2, 3, 6, 7).reshape(H, W, 81)
    return np.ascontiguousarray(out).astype(np.float32)


def kernel(feat1: np.ndarray, feat2: np.ndarray) -> np.ndarray:
    global _CACHED_NC
    feat1 = np.ascontiguousarray(np.asarray(feat1), dtype=np.float32)
    feat2 = np.ascontiguousarray(np.asarray(feat2), dtype=np.float32)
    B = feat1.shape[0]
    if _CACHED_NC is None:
        _CACHED_NC = _build()
    nc = _CACHED_NC
    in_maps = [{"feat1": feat1[b], "feat2": feat2[b]} for b in range(B)]
    res = run_bass_kernel_spmd(nc, in_maps, core_ids=list(range(B)))
    out = np.stack([_extract_host(res.results[b]["out"]) for b in range(B)],
                   axis=0)
    return out
